# revision 1
# baseline (speedup 1.0000x reference)
"""MoE grouped-expert SwiGLU MLP kernel for 8 Trainium2 NeuronCores.

Problem: x[T=32768, D=4096] routed to E=8 experts (packed rows, counts in
num_tokens_per_expert), per-expert SwiGLU MLP with w1/w3 [E, D, I=1024] and
w2 [E, I, D], bf16 compute, f32 output.

Strategy: expert parallelism, one expert per core, zero collectives.
Core c gets the token rows of expert c (host-sliced) plus expert c's weights,
computes out_c = (silu(x_c @ w1_c) * (x_c @ w3_c)) @ w2_c, and the host
concatenates the 8 output slices.

Per-core dataflow (all device GEMMs in bf16, f32 PSUM accumulation):
  - activations live transposed: xT [D, TC] so the contraction dim (D) is on
    SBUF partitions for GEMM1.
  - GEMM1: stationary w1/w3 tiles [d128, i128] (resident in SBUF), moving
    xT [d128, tok512] -> psum x1T/x3T [i128, tok512].
  - SwiGLU: silu(psum1) on ACT, * psum3 on DVE -> hT [i, tok] bf16 in SBUF.
  - GEMM2: stationary hT [i128, tok128], moving w2 [i128, d512] (streamed)
    -> psum out [tok128, d512] -> bf16 -> DMA to out[TC, D] (natural layout).
"""

import os
import sys

import numpy as np
import ml_dtypes

for _p in ("/opt/trn_rl_repo", "/root/.axon_site", "/root/.axon_site/_ro/trn_rl_repo"):
    if os.path.isdir(_p) and _p not in sys.path:
        sys.path.append(_p)

E, D, I, T = 8, 4096, 1024, 32768
N_CORES = 8

_BUILD_CACHE = {}


def build_core_kernel(d=D, i_dim=I, tc_tokens=T // N_CORES, tokb=512):
    """Build + compile the single-core Bass program (SPMD across 8 cores)."""
    import concourse.bacc as bacc
    import concourse.tile as tile
    import concourse.mybir as mybir

    key = (d, i_dim, tc_tokens, tokb)
    if key in _BUILD_CACHE:
        return _BUILD_CACHE[key]

    bf16 = mybir.dt.bfloat16
    f32 = mybir.dt.float32

    ND = d // 128          # contraction tiles for GEMM1
    NI = i_dim // 128      # intermediate tiles
    NB = tc_tokens // tokb  # token blocks
    NTK = tokb // 128      # 128-token subtiles per block
    DJ = min(512, d)       # GEMM2 output column tile
    NDJ = d // DJ

    nc = bacc.Bacc("TRN2", debug=False, target_bir_lowering=False,
                   num_devices=N_CORES)

    xT = nc.dram_tensor("xt_in", [d, tc_tokens], bf16, kind="ExternalInput").ap()
    w1 = nc.dram_tensor("w1_in", [d, i_dim], bf16, kind="ExternalInput").ap()
    w3 = nc.dram_tensor("w3_in", [d, i_dim], bf16, kind="ExternalInput").ap()
    w2 = nc.dram_tensor("w2_in", [i_dim, d], bf16, kind="ExternalInput").ap()
    out = nc.dram_tensor("out_res", [tc_tokens, d], bf16, kind="ExternalOutput").ap()

    xTv = xT.rearrange("(dt p) t -> dt p t", p=128)
    w1v = w1.rearrange("(dt p) i -> dt p i", p=128)
    w3v = w3.rearrange("(dt p) i -> dt p i", p=128)
    w2v = w2.rearrange("(it p) dd -> it p dd", p=128)

    with tile.TileContext(nc) as tc:
        with (
            tc.tile_pool(name="wres", bufs=1) as wres,
            tc.tile_pool(name="xtp", bufs=1) as xtp,
            tc.tile_pool(name="htp", bufs=1) as htp,
            tc.tile_pool(name="w2p", bufs=3) as w2p,
            tc.tile_pool(name="evac", bufs=3) as evac,
            tc.tile_pool(name="ostg", bufs=4) as ostg,
            tc.tile_pool(name="ps1", bufs=2, space="PSUM") as ps1,
            tc.tile_pool(name="ps3", bufs=2, space="PSUM") as ps3,
            tc.tile_pool(name="pso", bufs=3, space="PSUM") as pso,
        ):
            # resident GEMM1 weights: one tile per 128-row d-slice so matmuls
            # only wait on the slices they read (fine-grained deps), and the
            # first block's GEMM1 can stream behind the preload DMA.
            w1sb = [wres.tile([128, i_dim], bf16, tag=f"w1_{dt}", name=f"w1_{dt}")
                    for dt in range(ND)]
            w3sb = [wres.tile([128, i_dim], bf16, tag=f"w3_{dt}", name=f"w3_{dt}")
                    for dt in range(ND)]
            xtsb = [None] * ND

            def load_xt(b):
                t0 = b * tokb
                for dt in range(ND):
                    xtsb[dt] = xtp.tile([128, tokb], bf16, tag=f"xt_{dt}", name=f"xt_{dt}")
                    nc.sync.dma_start(xtsb[dt][:], xTv[dt, :, t0:t0 + tokb])

            # interleave weight + first-block xT loads d-slice by d-slice so
            # the i=0 accumulation pass can start almost immediately
            t0 = 0
            for dt in range(ND):
                nc.sync.dma_start(w1sb[dt][:], w1v[dt])
                nc.sync.dma_start(w3sb[dt][:], w3v[dt])
                xtsb[dt] = xtp.tile([128, tokb], bf16, tag=f"xt_{dt}", name=f"xt_{dt}")
                nc.sync.dma_start(xtsb[dt][:], xTv[dt, :, t0:t0 + tokb])

            def load_w2(dj):
                c0 = dj * DJ
                w2sb = w2p.tile([128, NI, DJ], bf16, tag="w2")
                for it in range(NI):
                    nc.sync.dma_start(w2sb[:, it, :], w2v[it, :, c0:c0 + DJ])
                return w2sb

            for b in range(NB):
                t0 = b * tokb
                if b > 0:
                    load_xt(b)
                xts = list(xtsb)

                w2_next = load_w2(0)  # prefetch during GEMM1
                htsb = [htp.tile([128, tokb], bf16, tag=f"ht_{it}", name=f"ht_{it}")
                        for it in range(NI)]
                for it in range(NI):
                    i0 = it * 128
                    p1 = ps1.tile([128, tokb], f32, tag="p1")
                    p3 = ps3.tile([128, tokb], f32, tag="p3")
                    for dt in range(ND):
                        nc.tensor.matmul(p1[:], w1sb[dt][:, i0:i0 + 128],
                                         xts[dt][:],
                                         start=(dt == 0), stop=(dt == ND - 1))
                    for dt in range(ND):
                        nc.tensor.matmul(p3[:], w3sb[dt][:, i0:i0 + 128],
                                         xts[dt][:],
                                         start=(dt == 0), stop=(dt == ND - 1))
                    sil = evac.tile([128, tokb], bf16, tag="sil")
                    nc.scalar.activation(sil[:], p1[:],
                                         mybir.ActivationFunctionType.Silu)
                    nc.vector.tensor_mul(htsb[it][:], sil[:], p3[:])

                for dj in range(NDJ):
                    c0 = dj * DJ
                    w2sb = w2_next
                    if dj + 1 < NDJ:
                        w2_next = load_w2(dj + 1)
                    for tk in range(NTK):
                        k0 = tk * 128
                        po = pso.tile([128, DJ], f32, tag="po")
                        for it in range(NI):
                            nc.tensor.matmul(po[:], htsb[it][:, k0:k0 + 128],
                                             w2sb[:, it, :],
                                             start=(it == 0), stop=(it == NI - 1))
                        og = ostg.tile([128, DJ], bf16, tag="og")
                        nc.vector.tensor_copy(og[:], po[:])
                        nc.sync.dma_start(
                            out[t0 + k0:t0 + k0 + 128, c0:c0 + DJ], og[:])

    nc.compile()
    _BUILD_CACHE[key] = nc
    return nc


def _run_cores(in_maps, d, i_dim, tc_tokens, tokb=512, trace=False):
    from concourse.bass_utils import run_bass_kernel_spmd

    nc = build_core_kernel(d, i_dim, tc_tokens, tokb)
    res = run_bass_kernel_spmd(nc, in_maps, core_ids=list(range(N_CORES)),
                               trace=trace)
    return res


def kernel(x, w1, w2, w3, num_tokens_per_expert, _trace=False, _ret_perf=None):
    x = np.asarray(x)
    w1 = np.asarray(w1)
    w2 = np.asarray(w2)
    w3 = np.asarray(w3)
    counts = np.asarray(num_tokens_per_expert).astype(np.int64)
    e, d, i_dim = w1.shape
    t = x.shape[0]
    assert e == N_CORES, f"expected {N_CORES} experts, got {e}"
    offs = np.concatenate([[0], np.cumsum(counts)])
    assert offs[-1] == t, f"token counts {counts} do not sum to {t}"

    bf = ml_dtypes.bfloat16
    # pad every expert group to a common multiple-of-512 token count so one
    # SPMD program serves all cores
    tc_tokens = max(512, int(-(-counts.max() // 512) * 512))
    tokb = 512

    xb = x.astype(bf)
    w1b = w1.astype(bf)
    w2b = w2.astype(bf)
    w3b = w3.astype(bf)

    in_maps = []
    for c in range(N_CORES):
        n = int(counts[c])
        xc = xb[offs[c]:offs[c] + n]
        if n < tc_tokens:
            pad = np.zeros((tc_tokens - n, d), dtype=bf)
            xc = np.concatenate([xc, pad], axis=0)
        in_maps.append({
            "xt_in": np.ascontiguousarray(xc.T),
            "w1_in": np.ascontiguousarray(w1b[c]),
            "w3_in": np.ascontiguousarray(w3b[c]),
            "w2_in": np.ascontiguousarray(w2b[c]),
        })

    res = _run_cores(in_maps, d, i_dim, tc_tokens, tokb, trace=_trace)
    if _ret_perf is not None:
        _ret_perf.append(res)

    out = np.empty((t, d), dtype=x.dtype)
    for c in range(N_CORES):
        n = int(counts[c])
        out[offs[c]:offs[c] + n] = res.results[c]["out_res"][:n].astype(x.dtype)
    return out



# revision 2
# speedup vs baseline: 1.0531x; 1.0531x over previous
"""MoE grouped-expert SwiGLU MLP kernel for 8 Trainium2 NeuronCores.

Problem: x[T=32768, D=4096] routed to E=8 experts (packed rows, counts in
num_tokens_per_expert), per-expert SwiGLU MLP with w1/w3 [E, D, I=1024] and
w2 [E, I, D], bf16 compute, f32 output.

Strategy: expert parallelism, one expert per core, zero collectives.
Core c gets the token rows of expert c (host-sliced) plus expert c's weights,
computes out_c = (silu(x_c @ w1_c) * (x_c @ w3_c)) @ w2_c, and the host
concatenates the 8 output slices.

Per-core dataflow (all device GEMMs in bf16, f32 PSUM accumulation):
  - w1/w3 resident in SBUF in it-major layout ([NI, 128, ND*128] in HBM) so
    the first it-group's weights arrive in one 1MB DMA and the PE can start
    ~15us into the kernel instead of waiting for the full 16MB preload.
  - activations live transposed: xT tile [128, ND, tokb] per token block,
    loaded in 4 x 1MB chunk DMAs, prefetched one block ahead (the prefetch is
    emitted early in the GEMM2 phase so it lands ~35us before it is needed).
  - GEMM1: stationary w1/w3 col-slices [128, 128], moving xT [128, tokb]
    -> psum x1T/x3T [128 i, tokb]; silu on ACT, * on DVE -> hT bf16.
  - GEMM2: stationary hT [128 i, 128 tok], moving w2 [128 i, DJ] (streamed
    dj-major, 4-deep prefetch) -> psum out [tok, DJ] (pso bufs=4: all 8 PSUM
    banks in use) -> DVE copy bf16 -> DMA to out[TC, D] on the ACT HWDGE
    queue so stores never sit ahead of loads on the SP queue.
"""

import os
import sys

import numpy as np
import ml_dtypes

for _p in ("/opt/trn_rl_repo", "/root/.axon_site", "/root/.axon_site/_ro/trn_rl_repo"):
    if os.path.isdir(_p) and _p not in sys.path:
        sys.path.append(_p)

E, D, I, T = 8, 4096, 1024, 32768
N_CORES = 8

_BUILD_CACHE = {}


def build_core_kernel(d=D, i_dim=I, tc_tokens=T // N_CORES, tokb=512):
    """Build + compile the single-core Bass program (SPMD across 8 cores)."""
    import concourse.bacc as bacc
    import concourse.tile as tile
    import concourse.mybir as mybir

    key = (d, i_dim, tc_tokens, tokb)
    if key in _BUILD_CACHE:
        return _BUILD_CACHE[key]

    bf16 = mybir.dt.bfloat16
    f32 = mybir.dt.float32

    ND = d // 128           # contraction tiles for GEMM1
    NI = i_dim // 128       # intermediate tiles
    NB = tc_tokens // tokb  # token blocks
    NTK = tokb // 128       # 128-token subtiles per block
    DJ = min(512, d)        # GEMM2 output column tile
    NDJ = d // DJ
    NXC = 4                 # xt chunk DMAs per block
    XC = ND // NXC          # dt slices per chunk

    nc = bacc.Bacc("TRN2", debug=False, target_bir_lowering=False,
                   num_devices=N_CORES)

    # it-major weights: w1_in[it, p, dt*128+m] = w1[dt*128+p, it*128+m]
    x_in = nc.dram_tensor("x_in", [NB, 128, ND, tokb], bf16,
                          kind="ExternalInput").ap()
    w1 = nc.dram_tensor("w1_in", [NI, 128, ND * 128], bf16,
                        kind="ExternalInput").ap()
    w3 = nc.dram_tensor("w3_in", [NI, 128, ND * 128], bf16,
                        kind="ExternalInput").ap()
    # dj-major w2: w2_in[dj, p, it*DJ+c] = w2[it*128+p, dj*DJ+c]
    w2 = nc.dram_tensor("w2_in", [NDJ, 128, NI * DJ], bf16,
                        kind="ExternalInput").ap()
    out = nc.dram_tensor("out_res", [tc_tokens, d], bf16,
                         kind="ExternalOutput").ap()

    NW2 = NB * NDJ  # total w2 dj-tile loads

    with tile.TileContext(nc) as tc:
        with (
            tc.tile_pool(name="wres", bufs=1) as wres,
            tc.tile_pool(name="xtp", bufs=1) as xtp,
            tc.tile_pool(name="htp", bufs=1) as htp,
            tc.tile_pool(name="w2p", bufs=4) as w2p,
            tc.tile_pool(name="evac", bufs=2) as evac,
            tc.tile_pool(name="ostg", bufs=3) as ostg,
            tc.tile_pool(name="ps1", bufs=2, space="PSUM") as ps1,
            tc.tile_pool(name="ps3", bufs=2, space="PSUM") as ps3,
            tc.tile_pool(name="pso", bufs=4, space="PSUM") as pso,
        ):
            w1sb = [wres.tile([128, ND * 128], bf16, tag=f"w1_{it}",
                              name=f"w1_{it}") for it in range(NI)]
            w3sb = [wres.tile([128, ND * 128], bf16, tag=f"w3_{it}",
                              name=f"w3_{it}") for it in range(NI)]

            xt_cur = [None]

            def load_xt(b):
                xt = xtp.tile([128, ND, tokb], bf16, tag="xt", name="xt")
                for c in range(NXC):
                    nc.sync.dma_start(xt[:, c * XC:(c + 1) * XC, :],
                                      x_in[b, :, c * XC:(c + 1) * XC, :])
                return xt

            w2q = []  # fifo of loaded w2 tiles

            def load_w2(g):
                b, dj = divmod(g, NDJ)
                w2sb = w2p.tile([128, NI, DJ], bf16, tag="w2")
                nc.sync.dma_start(w2sb[:], w2[dj])
                w2q.append(w2sb)

            # startup: first it-group weights + first token block first, so
            # the PE can start after ~5MB instead of the full 20MB preload
            nc.sync.dma_start(w1sb[0][:], w1[0])
            xt_next = load_xt(0)
            nc.sync.dma_start(w3sb[0][:], w3[0])
            for it in range(1, NI):
                nc.sync.dma_start(w1sb[it][:], w1[it])
                nc.sync.dma_start(w3sb[it][:], w3[it])
            for g in range(3):
                load_w2(g)

            for b in range(NB):
                t0 = b * tokb
                xts = xt_next

                htsb = [htp.tile([128, tokb], bf16, tag=f"ht_{it}",
                                 name=f"ht_{it}") for it in range(NI)]
                for it in range(NI):
                    i0 = it * 128
                    p1 = ps1.tile([128, tokb], f32, tag="p1")
                    p3 = ps3.tile([128, tokb], f32, tag="p3")
                    for dt in range(ND):
                        nc.tensor.matmul(p1[:],
                                         w1sb[it][:, dt * 128:dt * 128 + 128],
                                         xts[:, dt, :],
                                         start=(dt == 0), stop=(dt == ND - 1))
                    for dt in range(ND):
                        nc.tensor.matmul(p3[:],
                                         w3sb[it][:, dt * 128:dt * 128 + 128],
                                         xts[:, dt, :],
                                         start=(dt == 0), stop=(dt == ND - 1))
                    sil = evac.tile([128, tokb], bf16, tag="sil")
                    nc.scalar.activation(sil[:], p1[:],
                                         mybir.ActivationFunctionType.Silu)
                    nc.vector.tensor_mul(htsb[it][:], sil[:], p3[:])

                for dj in range(NDJ):
                    g_pref = b * NDJ + dj + 3
                    if g_pref < NW2:
                        load_w2(g_pref)
                    if dj == 1 and b + 1 < NB:
                        xt_next = load_xt(b + 1)
                    c0 = dj * DJ
                    w2sb = w2q.pop(0)
                    for tk in range(NTK):
                        k0 = tk * 128
                        po = pso.tile([128, DJ], f32, tag="po")
                        for it in range(NI):
                            nc.tensor.matmul(po[:], htsb[it][:, k0:k0 + 128],
                                             w2sb[:, it, :],
                                             start=(it == 0),
                                             stop=(it == NI - 1))
                        og = ostg.tile([128, DJ], bf16, tag="og")
                        nc.vector.tensor_copy(og[:], po[:])
                        nc.scalar.dma_start(
                            out[t0 + k0:t0 + k0 + 128, c0:c0 + DJ], og[:])

    nc.compile()
    _BUILD_CACHE[key] = nc
    return nc


def _run_cores(in_maps, d, i_dim, tc_tokens, tokb=512, trace=False):
    from concourse.bass_utils import run_bass_kernel_spmd

    nc = build_core_kernel(d, i_dim, tc_tokens, tokb)
    res = run_bass_kernel_spmd(nc, in_maps, core_ids=list(range(N_CORES)),
                               trace=trace)
    return res


def kernel(x, w1, w2, w3, num_tokens_per_expert, _trace=False, _ret_perf=None):
    x = np.asarray(x)
    w1 = np.asarray(w1)
    w2 = np.asarray(w2)
    w3 = np.asarray(w3)
    counts = np.asarray(num_tokens_per_expert).astype(np.int64)
    e, d, i_dim = w1.shape
    t = x.shape[0]
    assert e == N_CORES, f"expected {N_CORES} experts, got {e}"
    offs = np.concatenate([[0], np.cumsum(counts)])
    assert offs[-1] == t, f"token counts {counts} do not sum to {t}"

    bf = ml_dtypes.bfloat16
    # pad every expert group to a common multiple-of-512 token count so one
    # SPMD program serves all cores
    tokb = 512
    tc_tokens = max(tokb, int(-(-counts.max() // tokb) * tokb))
    NB = tc_tokens // tokb
    ND = d // 128
    NI = i_dim // 128
    DJ = min(512, d)
    NDJ = d // DJ

    w1b = w1.astype(bf)
    w2b = w2.astype(bf)
    w3b = w3.astype(bf)

    in_maps = []
    for c in range(N_CORES):
        n = int(counts[c])
        xc = x[offs[c]:offs[c] + n].astype(bf)
        if n < tc_tokens:
            pad = np.zeros((tc_tokens - n, d), dtype=bf)
            xc = np.concatenate([xc, pad], axis=0)
        # x_in[b, p, dt, tt] = xc[b*tokb+tt, dt*128+p]
        xr = np.ascontiguousarray(
            xc.reshape(NB, tokb, ND, 128).transpose(0, 3, 2, 1))
        # w1_in[it, p, dt*128+m] = w1[dt*128+p, it*128+m]
        w1r = np.ascontiguousarray(
            w1b[c].reshape(ND, 128, NI, 128).transpose(2, 1, 0, 3)
            .reshape(NI, 128, ND * 128))
        w3r = np.ascontiguousarray(
            w3b[c].reshape(ND, 128, NI, 128).transpose(2, 1, 0, 3)
            .reshape(NI, 128, ND * 128))
        # w2_in[dj, p, it*DJ+cc] = w2[it*128+p, dj*DJ+cc]
        w2r = np.ascontiguousarray(
            w2b[c].reshape(NI, 128, NDJ, DJ).transpose(2, 1, 0, 3)
            .reshape(NDJ, 128, NI * DJ))
        in_maps.append({
            "x_in": xr,
            "w1_in": w1r,
            "w3_in": w3r,
            "w2_in": w2r,
        })

    res = _run_cores(in_maps, d, i_dim, tc_tokens, tokb, trace=_trace)
    if _ret_perf is not None:
        _ret_perf.append(res)

    out = np.empty((t, d), dtype=x.dtype)
    for c in range(N_CORES):
        n = int(counts[c])
        out[offs[c]:offs[c] + n] = res.results[c]["out_res"][:n].astype(x.dtype)
    return out
